# revision 23
# baseline (speedup 1.0000x reference)
"""Causal self-attention (B=4, T=2048, C=1024, 16 heads) on 8 trn2 NeuronCores.

Sharding: core c = (batch c//2, head-group c%2 of 8 heads). Data-parallel over
batch, tensor-parallel over heads; out-proj is row-sharded and the two partial
products per batch are summed on the host (no device collectives).

Fused software pipeline over 4 t-chunks of 512: within chunk t the attention
spine (S^T matmul -> exp -> P@V) for qc=t is interleaved on the PE queue with
"filler" psum groups: qkv projections for chunk t+1 and (during the last
chunk) the out-projections of chunks 0-2. All matmul operands are bf16
(converted host-side); softmax stats stay fp32; the output is written bf16
and the two partial products are summed fp32 on the host.

v2 changes vs the first working version:
- P@V deferred by TWO ki steps (was one) so the exp->mask chain never stalls
  the PE during boundary-heavy phases.
- All host inputs repacked to partition-major [128, .] contiguous layouts:
  single large DMAs with >=2KB descriptors (no read-modify-write penalty).
- Output in bf16; chunks 0-2 staged in SBUF and written with one DMA per
  chunk; last chunk per-group to shorten the drain tail.
- PE warm-up matmuls at t=0 keep the tensor-engine p-state ramp continuous
  until real operands arrive.
- Normalization flush fuses the yq copy into the 1/denom multiply.
"""

import os
import sys

import numpy as np

for _p in ("/opt/trn_rl_repo", "/root/.axon_site/_ro/trn_rl_repo"):
    if os.path.isdir(_p) and _p not in sys.path:
        sys.path.insert(0, _p)

import concourse.bass as bass  # noqa: E402
import concourse.tile as tile  # noqa: E402
from concourse import bacc, mybir  # noqa: E402
from concourse.bass_utils import run_bass_kernel_spmd  # noqa: E402

B, T, C = 4, 2048, 1024
H, D = 16, 64
N_CORES = 8
F32 = mybir.dt.float32
BF16 = mybir.dt.bfloat16

TC = T // 512  # 4 t-chunks of 512
TT = T // 128  # 16 t-tiles of 128
CT = C // 128  # 8 c-tiles of 128

_cache: dict = {}


def _emit(nc: "bacc.Bacc", tc: "tile.TileContext", d: dict) -> None:
    mult = mybir.AluOpType.mult
    add = mybir.AluOpType.add
    Exp = mybir.ActivationFunctionType.Exp
    dma = nc.sync.dma_start

    with (
        tc.tile_pool(name="const", bufs=1) as cpool,
        tc.tile_pool(name="persist", bufs=1) as persist,
        tc.tile_pool(name="wqkv", bufs=1) as wpool,
        tc.tile_pool(name="xt1", bufs=2) as xpool,
        tc.tile_pool(name="qc", bufs=2) as qpool,
        tc.tile_pool(name="yc", bufs=4) as ypool,
        tc.tile_pool(name="es", bufs=6) as espool,
        tc.tile_pool(name="rc", bufs=3) as rcpool,
        tc.tile_pool(name="rcl", bufs=1) as rclpool,
        tc.tile_pool(name="osb", bufs=3) as opool,
        tc.tile_pool(name="stg", bufs=3) as stpool,
        tc.tile_pool(name="fps", bufs=2, space="PSUM") as fps,
        tc.tile_pool(name="pss", bufs=2, space="PSUM") as pss,
        tc.tile_pool(name="psy", bufs=1, space="PSUM") as psy,
    ):
        # --- PE warm-up: dependency-free matmuls keep the p-state ramp hot
        # from t~=0 until the first real operands land.
        wtile = cpool.tile([128, 512], BF16, name="wtile", tag="wtile")
        nc.vector.memset(wtile[:, 0:128], 0.0)
        warm_ps = pss.tile([128, 1024], F32, name="wps", tag="sblk")

        def warm(n):
            for _ in range(n):
                nc.tensor.matmul(
                    warm_ps[:, 0:256],
                    wtile[:, 0:128],
                    wtile[:, 0:256],
                    start=True,
                    stop=True,
                    skip_group_check=True,
                )

        warm(16)

        warm_sb = cpool.tile([128, 2], F32, name="warm", tag="warm")
        nc.vector.memset(warm_sb[:, 0:1], 0.0)
        nc.scalar.activation(warm_sb[:, 1:2], warm_sb[:, 0:1], Exp)

        m01_sb = cpool.tile([128, 128], BF16, name="m01", tag="m01")
        cs_sb = cpool.tile([128, 528], F32, name="cs", tag="cs")
        bq_sb = cs_sb[:, 0:4]
        bk_sb = cs_sb[:, 4:8]
        bv_sb = cs_sb[:, 8:528]

        kT = [persist.tile([128, T], BF16, name=f"kT{p}", tag=f"kT{p}") for p in range(4)]
        Vt = [persist.tile([128, 520], BF16, name=f"V{i}", tag=f"V{i}") for i in range(TT)]

        def pv_step(es, ki, base, yqs, p, nki):
            r = ki - base
            vs = max(0, 128 * r)
            first, last = ki == 0, ki == nki - 1
            for h2 in (0, 1):
                hl = 2 * p + h2
                nc.tensor.matmul(
                    yqs[h2][:, vs:512],
                    Vt[ki][:, 65 * hl : 65 * hl + 65],
                    es[:, 512 * h2 + vs : 512 * h2 + 512],
                    start=first,
                    stop=last,
                    skip_group_check=True,
                )

        wq_sb = wpool.tile([128, CT, 512], BF16, name="wq", tag="wq")
        wk_sb = wpool.tile([128, CT, 512], BF16, name="wk", tag="wk")
        wv_sb = wpool.tile([128, CT, 520], BF16, name="wv", tag="wv")
        wp_sb = wpool.tile([128, 4, 1024], BF16, name="wp", tag="wp")

        def dma_x(t):
            xt = xpool.tile([128, CT, 512], BF16, name="xt", tag="xt")
            dma(out=xt[:], in_=d["x8"].ap()[:, 4096 * t : 4096 * t + 4096])
            return xt

        # chunk-0 inputs in quarter-chunks so the first psum group's matmuls
        # can start while the rest streams; consts (biases) must land before
        # the first q/k groups' tensor_scalar consumers
        xt0 = xpool.tile([128, CT, 512], BF16, name="xt", tag="xt")
        for qtr in range(4):
            lo, hi = 1024 * qtr, 1024 * qtr + 1024
            dma(out=wq_sb[:, 2 * qtr : 2 * qtr + 2, :], in_=d["wq"].ap()[:, lo:hi])
            dma(out=xt0[:, 2 * qtr : 2 * qtr + 2, :], in_=d["x8"].ap()[:, lo:hi])
        dma(out=cs_sb[:], in_=d["consts"].ap())
        dma(out=wk_sb[:], in_=d["wk"].ap())
        dma(out=wv_sb[:], in_=d["wv"].ap())
        xts = {0: xt0, 1: dma_x(1)}
        dma(out=m01_sb[:], in_=d["m01"].ap())
        dma(out=wp_sb[:], in_=d["wp"].ap())

        qTc = {}  # chunk -> [4 tiles of [128,512] bf16]
        yTc = {}  # chunk -> [4 tiles of [128,512] bf16]
        stage = {}  # chunk -> [128, 4, 1024] bf16 staged out rows

        # ---- filler group emitters (each closure emits one psum group) ----
        def qk_group(t, p, iw, w_sb=None, mid=None):
            def g(w_sb=w_sb):
                xt = xts[t]
                ps = fps.tile([128, 512], F32, name="fps", tag="fps")
                for ct in range(CT):
                    nc.tensor.matmul(
                        ps[:],
                        w_sb[:, ct, 128 * p : 128 * p + 128],
                        xt[:, ct, :],
                        start=(ct == 0),
                        stop=(ct == CT - 1),
                        skip_group_check=True,
                    )
                    if mid is not None and ct in (1, 3, 5):
                        mid()
                if iw == 0:
                    nc.vector.tensor_scalar(
                        qTc[t][p][:],
                        ps[:],
                        0.125,
                        bq_sb[:, p : p + 1],
                        mult,
                        add,
                    )
                else:
                    nc.vector.tensor_scalar(
                        kT[p][:, 512 * t : 512 * t + 512],
                        ps[:],
                        1.0,
                        bk_sb[:, p : p + 1],
                        mult,
                        add,
                    )

            return g

        def v_group(t, tt, qd):
            def g():
                xt = xts[t]
                ps = fps.tile([128, 512], F32, name="fps", tag="fps")
                for ct in range(CT):
                    nc.tensor.matmul(
                        ps[:, 0:260],
                        xt[:, ct, 128 * tt : 128 * tt + 128],
                        wv_sb[:, ct, 260 * qd : 260 * qd + 260],
                        start=(ct == 0),
                        stop=(ct == CT - 1),
                        skip_group_check=True,
                    )
                nc.vector.tensor_tensor(
                    Vt[4 * t + tt][:, 260 * qd : 260 * qd + 260],
                    ps[:, 0:260],
                    bv_sb[:, 260 * qd : 260 * qd + 260],
                    add,
                )

            return g

        def qkv_groups(t, first=False):
            gs = []
            for p in range(4):
                mid = (lambda: warm(5)) if (first and p == 0) else None
                gs.append(qk_group(t, p, 0, wq_sb, mid=mid))
            for p in range(4):
                gs.append(qk_group(t, p, 1, wk_sb))
            for tt in range(4):
                for qd in (0, 1):
                    gs.append(v_group(t, tt, qd))
            return gs

        def proj_group(t, ttl, cc, eng="act"):
            def g():
                ps = fps.tile([128, 512], F32, name="fps", tag="fps")
                for pp in range(4):
                    nc.tensor.matmul(
                        ps[:],
                        yTc[t][pp][:, 128 * ttl : 128 * ttl + 128],
                        wp_sb[:, pp, 512 * cc : 512 * cc + 512],
                        start=(pp == 0),
                        stop=(pp == 3),
                        skip_group_check=True,
                    )
                if t == TC - 1:
                    # epilogue: per-group DMA keeps the drain tail short
                    row = 512 * t + 128 * ttl
                    ob = opool.tile([128, 512], BF16, name="ob", tag="ob")
                    if eng == "act":
                        nc.scalar.copy(ob[:], ps[:])
                    else:
                        nc.vector.tensor_copy(out=ob[:], in_=ps[:])
                    dma(
                        out=d["out"].ap()[row : row + 128, 512 * cc : 512 * cc + 512],
                        in_=ob[:],
                    )
                    return
                if t not in stage:
                    stage[t] = stpool.tile(
                        [128, 4, 1024], BF16, name=f"st{t}", tag="st"
                    )
                st = stage[t]
                if eng == "act":
                    nc.scalar.copy(st[:, ttl, 512 * cc : 512 * cc + 512], ps[:])
                else:
                    nc.vector.tensor_copy(
                        out=st[:, ttl, 512 * cc : 512 * cc + 512], in_=ps[:]
                    )
                if ttl == 3 and cc == 1:
                    dma(
                        out=d["out"]
                        .ap()[512 * t : 512 * t + 512, :]
                        .rearrange("(tt p) n -> p tt n", p=128),
                        in_=st[:],
                    )

            return g

        def proj_groups(t, eng="act"):
            return [
                proj_group(t, ttl, cc, eng)
                for ttl in range(4)
                for cc in (0, 1)
            ]

        def flush_tail(yq_t, p, t, sliced=False):
            if not sliced:
                for h2 in (0, 1):
                    pr = 64 * h2
                    rr = rcpool.tile([1, 512], F32, name="rr", tag="rr")
                    nc.vector.reciprocal(rr[:], yq_t[h2][64:65, :])
                    rcb = rcpool.tile([128, 512], F32, name="rcb", tag="rcb")
                    nc.gpsimd.partition_broadcast(rcb[:], rr[:])
                    nc.vector.tensor_tensor(
                        yTc[t][p][pr : pr + 64, :],
                        yq_t[h2][0:64, :],
                        rcb[pr : pr + 64, :],
                        mult,
                    )
                return
            # final flush: 128-col slices so the first epilogue out-proj
            # group starts as soon as its slice is normalized
            rcbs = [
                rclpool.tile([128, 512], F32, name=f"rcb{h2}", tag=f"rcbs{h2}")
                for h2 in (0, 1)
            ]
            rrs = [
                rclpool.tile([1, 512], F32, name=f"rrs{h2}", tag=f"rrs{h2}")
                for h2 in (0, 1)
            ]
            for sl in range(4):
                c0 = 128 * sl
                for h2 in (0, 1):
                    pr = 64 * h2
                    nc.vector.reciprocal(
                        rrs[h2][:, c0 : c0 + 128], yq_t[h2][64:65, c0 : c0 + 128]
                    )
                    nc.gpsimd.partition_broadcast(
                        rcbs[h2][:, c0 : c0 + 128], rrs[h2][:, c0 : c0 + 128]
                    )
                    nc.vector.tensor_tensor(
                        yTc[t][p][pr : pr + 64, c0 : c0 + 128],
                        yq_t[h2][0:64, c0 : c0 + 128],
                        rcbs[h2][pr : pr + 64, c0 : c0 + 128],
                        mult,
                    )

        # ---- prologue: qkv for chunk 0 ----
        qTc[0] = [
            qpool.tile([128, 512], BF16, name=f"qT{p}", tag=f"qT{p}") for p in range(4)
        ]
        for g in qkv_groups(0, first=True):
            g()

        # ---- fused chunk pipeline ----
        for t in range(TC):
            # stage next chunks' inputs and allocate next-gen tiles
            if t + 2 < TC:
                xts[t + 2] = dma_x(t + 2)
            filler = []
            if t + 1 < TC:
                qTc[t + 1] = [
                    qpool.tile([128, 512], BF16, name=f"qT{p}", tag=f"qT{p}")
                    for p in range(4)
                ]
                filler += qkv_groups(t + 1)
            # all mid-stream out-proj filler rides in the last chunk, where
            # the exp load is heaviest and the spine alone cannot feed PE
            if t == 3:
                filler += (
                    proj_groups(0, eng="dve")
                    + proj_groups(1, eng="dve")
                    + proj_groups(2, eng="dve")
                )
            yTc[t] = [
                ypool.tile([128, 512], BF16, name=f"yT{p}", tag=f"yT{p}")
                for p in range(4)
            ]

            nki = 4 * t + 4
            nsteps = 4 * nki
            quota = len(filler) / nsteps
            acc = 0.0
            si = 0
            fi = 0
            pending = None  # (yqs, p) flush deferred to overlap next p's S work
            for p in range(4):
                yqs = [
                    psy.tile([65, 512], F32, name=f"yq{h2}", tag=f"yq{h2}")
                    for h2 in (0, 1)
                ]
                if pending is not None:
                    flush_tail(pending[0], pending[1], t)
                    pending = None
                    if fi < len(filler):
                        filler[fi]()
                        fi += 1
                        acc -= 1.0
                es_blk = [None] * nki
                for ki in range(nki):
                    r = ki - 4 * t
                    off = 128 * r if r >= 1 else 0
                    # S^T block (both heads in one psum tile), trimmed to the
                    # causally reachable q-columns
                    sblk = pss.tile([128, 1024], F32, name="sblk", tag="sblk")
                    for h2 in (0, 1):
                        pr = 64 * h2
                        nc.tensor.matmul(
                            sblk[:, 512 * h2 + off : 512 * h2 + 512],
                            kT[p][pr : pr + 64, 128 * ki : 128 * ki + 128],
                            qTc[t][p][pr : pr + 64, off:512],
                            start=True,
                            stop=True,
                        )
                    # deferred P@V of ki-2 keeps PE 2 steps ahead of the
                    # exp->mask chain
                    if ki >= 2:
                        pv_step(es_blk[ki - 2], ki - 2, 4 * t, yqs, p, nki)
                    # filler: qkv(t+1) / out-proj psum groups ride the gaps
                    si += 1
                    if t == TC - 1:
                        acc += quota * 2.0 * si / nsteps
                    else:
                        acc += quota
                    while acc >= 1.0 and fi < len(filler):
                        filler[fi]()
                        fi += 1
                        acc -= 1.0
                    es = espool.tile([128, 1024], BF16, name="es", tag="es")
                    if r >= 1:
                        # exp only the reachable region of each half, one
                        # instruction via a [512,2] middle dim over the halves
                        w = 512 - 128 * r
                        sv = sblk[:, 128 * r : 128 * r + w]
                        s2 = bass.AP(
                            tensor=sv.tensor,
                            offset=sv.offset,
                            ap=[list(sv.ap[0]), [512, 2], list(sv.ap[1])],
                        )
                        ev = es[:, 128 * r : 128 * r + w]
                        e2 = bass.AP(
                            tensor=ev.tensor,
                            offset=ev.offset,
                            ap=[list(ev.ap[0]), [512, 2], list(ev.ap[1])],
                        )
                        nc.scalar.activation(e2, s2, Exp)
                    else:
                        nc.scalar.activation(es[:], sblk[:], Exp)
                    if r >= 0:
                        for h2 in (0, 1):
                            c0 = 512 * h2 + 128 * r
                            nc.vector.tensor_tensor(
                                es[:, c0 : c0 + 128],
                                es[:, c0 : c0 + 128],
                                m01_sb[:],
                                mult,
                            )
                    es_blk[ki] = es
                pv_step(es_blk[nki - 2], nki - 2, 4 * t, yqs, p, nki)
                pv_step(es_blk[nki - 1], nki - 1, 4 * t, yqs, p, nki)
                pending = (yqs, p)
            flush_tail(pending[0], pending[1], t, sliced=(t == TC - 1))
            pending = None
            # drain any left-over filler
            while fi < len(filler):
                filler[fi]()
                fi += 1

        # ---- epilogue: out-projection of the last chunk ----
        for g in proj_groups(TC - 1):
            g()


def _build():
    nc = bacc.Bacc("TRN2", target_bir_lowering=False, debug=False, num_devices=N_CORES)
    d = {
        "x8": nc.dram_tensor("x8", [128, 4 * CT * 512], BF16, kind="ExternalInput"),
        "wq": nc.dram_tensor("wq", [128, CT * 512], BF16, kind="ExternalInput"),
        "wk": nc.dram_tensor("wk", [128, CT * 512], BF16, kind="ExternalInput"),
        "wv": nc.dram_tensor("wv", [128, CT * 520], BF16, kind="ExternalInput"),
        "consts": nc.dram_tensor("consts", [128, 528], F32, kind="ExternalInput"),
        "m01": nc.dram_tensor("m01", [128, 128], BF16, kind="ExternalInput"),
        "wp": nc.dram_tensor("wp", [128, 4 * 1024], BF16, kind="ExternalInput"),
        "out": nc.dram_tensor("out", [T, C], BF16, kind="ExternalOutput"),
    }
    with tile.TileContext(nc) as tcx:
        _emit(nc, tcx, d)
    nc.compile()
    return nc


def _prep_core_inputs(c, x, w_attn, b_attn):
    import ml_dtypes

    bf = ml_dtypes.bfloat16
    g = c % 2
    # x8[p, 4096*t + 512*ct + n] = x[b, 512*t + n, 128*ct + p]
    xT = np.ascontiguousarray(x[c // 2].T)  # [C, T]
    x8 = np.ascontiguousarray(
        xT.reshape(CT, 128, TC, 512).transpose(1, 2, 0, 3).reshape(128, -1)
    ).astype(bf)

    def pack_w(w):  # [C, n] -> [128, CT*n], ct-major per partition
        n = w.shape[1]
        return np.ascontiguousarray(
            w.reshape(CT, 128, n).transpose(1, 0, 2).reshape(128, -1)
        ).astype(bf)

    wq = pack_w(w_attn[:, 512 * g : 512 * g + 512])
    wk = pack_w(w_attn[:, 1024 + 512 * g : 1024 + 512 * g + 512])
    wv_f = np.zeros((C, 520), np.float32)
    consts = np.zeros((128, 528), np.float32)
    for hl in range(8):
        hcol = 2048 + 512 * g + 64 * hl
        wv_f[:, 65 * hl : 65 * hl + 64] = w_attn[:, hcol : hcol + 64]
        consts[:, 8 + 65 * hl : 8 + 65 * hl + 64] = b_attn[hcol : hcol + 64][None, :]
        consts[:, 8 + 65 * hl + 64] = 1.0
    wv = pack_w(wv_f)
    for p in range(4):
        consts[:, p] = b_attn[512 * g + 128 * p : 512 * g + 128 * p + 128] * 0.125
        consts[:, 4 + p] = b_attn[
            1024 + 512 * g + 128 * p : 1024 + 512 * g + 128 * p + 128
        ]
    m01 = (np.arange(128)[:, None] <= np.arange(128)[None, :]).astype(bf)
    return dict(x8=x8, wq=wq, wk=wk, wv=wv, consts=consts, m01=m01)


def make_in_maps(x, w_attn, b_attn, w_proj):
    import ml_dtypes

    bf = ml_dtypes.bfloat16
    x = np.asarray(x, np.float32)
    w_attn = np.asarray(w_attn, np.float32)
    b_attn = np.asarray(b_attn, np.float32)
    w_proj = np.asarray(w_proj, np.float32)
    in_maps = []
    for c in range(N_CORES):
        m = _prep_core_inputs(c, x, w_attn, b_attn)
        g = c % 2
        wp = w_proj[512 * g : 512 * g + 512, :]  # [512, 1024]
        m["wp"] = np.ascontiguousarray(
            wp.reshape(4, 128, 1024).transpose(1, 0, 2).reshape(128, -1)
        ).astype(bf)
        in_maps.append(m)
    return in_maps


def get_nc():
    if "nc" not in _cache:
        _cache["nc"] = _build()
    return _cache["nc"]


def gather(results, b_proj):
    b_proj = np.asarray(b_proj, np.float32)
    full = np.empty((B, T, C), np.float32)
    for b in range(B):
        full[b] = (
            results[2 * b]["out"].astype(np.float32)
            + results[2 * b + 1]["out"].astype(np.float32)
            + b_proj[None, :]
        )
    return full


def kernel(x, w_attn, b_attn, w_proj, b_proj):
    nc = get_nc()
    in_maps = make_in_maps(x, w_attn, b_attn, w_proj)
    res = run_bass_kernel_spmd(nc, in_maps, list(range(N_CORES)))
    return gather(res.results, b_proj)


# revision 37
# speedup vs baseline: 1.0269x; 1.0269x over previous
"""Causal self-attention (B=4, T=2048, C=1024, 16 heads) on 8 trn2 NeuronCores.

Sharding: core c = (batch c//2, head-group c%2 of 8 heads). Data-parallel over
batch, tensor-parallel over heads; out-proj is row-sharded and the two partial
products per batch are summed on the host (no device collectives).

Fused software pipeline over 4 t-chunks of 512: within chunk t the attention
spine (S^T matmul -> exp -> P@V) for qc=t is interleaved on the PE queue with
"filler" psum groups: qkv projections for chunk t+1 and (during the last
chunk) the out-projections of chunks 0-2. All matmul operands are bf16
(converted host-side); softmax stats stay fp32; the output is written bf16
and the two partial products are summed fp32 on the host.

v2 changes vs the first working version:
- P@V deferred by TWO ki steps (was one) so the exp->mask chain never stalls
  the PE during boundary-heavy phases.
- All host inputs repacked to partition-major [128, .] contiguous layouts:
  single large DMAs with >=2KB descriptors (no read-modify-write penalty).
- Output in bf16; chunks 0-2 staged in SBUF and written with one DMA per
  chunk; last chunk per-group to shorten the drain tail.
- PE warm-up matmuls at t=0 keep the tensor-engine p-state ramp continuous
  until real operands arrive.
- Normalization flush fuses the yq copy into the 1/denom multiply.
"""

import os
import sys

import numpy as np

for _p in ("/opt/trn_rl_repo", "/root/.axon_site/_ro/trn_rl_repo"):
    if os.path.isdir(_p) and _p not in sys.path:
        sys.path.insert(0, _p)

import concourse.bass as bass  # noqa: E402
import concourse.tile as tile  # noqa: E402
from concourse import bacc, mybir  # noqa: E402
from concourse.bass_utils import run_bass_kernel_spmd  # noqa: E402

B, T, C = 4, 2048, 1024
H, D = 16, 64
N_CORES = 8
F32 = mybir.dt.float32
BF16 = mybir.dt.bfloat16

TC = T // 512  # 4 t-chunks of 512
TT = T // 128  # 16 t-tiles of 128
CT = C // 128  # 8 c-tiles of 128

_cache: dict = {}


def _emit(nc: "bacc.Bacc", tc: "tile.TileContext", d: dict) -> None:
    mult = mybir.AluOpType.mult
    add = mybir.AluOpType.add
    Exp = mybir.ActivationFunctionType.Exp
    dma = nc.sync.dma_start

    with (
        tc.tile_pool(name="const", bufs=1) as cpool,
        tc.tile_pool(name="persist", bufs=1) as persist,
        tc.tile_pool(name="wqkv", bufs=1) as wpool,
        tc.tile_pool(name="xt1", bufs=2) as xpool,
        tc.tile_pool(name="qc", bufs=2) as qpool,
        tc.tile_pool(name="yc", bufs=4) as ypool,
        tc.tile_pool(name="es", bufs=6) as espool,
        tc.tile_pool(name="rc", bufs=3) as rcpool,
        tc.tile_pool(name="rcl", bufs=1) as rclpool,
        tc.tile_pool(name="osb", bufs=3) as opool,
        tc.tile_pool(name="stg", bufs=3) as stpool,
        tc.tile_pool(name="fps", bufs=2, space="PSUM") as fps,
        tc.tile_pool(name="pss", bufs=2, space="PSUM") as pss,
        tc.tile_pool(name="psy", bufs=1, space="PSUM") as psy,
    ):
        # --- PE warm-up: dependency-free matmuls keep the p-state ramp hot
        # from t~=0 until the first real operands land.
        wtile = cpool.tile([128, 512], BF16, name="wtile", tag="wtile")
        nc.gpsimd.memset(wtile[:, 0:128], 0.0)
        warm_ps = pss.tile([128, 1024], F32, name="wps", tag="sblk")

        def warm(n):
            for _ in range(n):
                nc.tensor.matmul(
                    warm_ps[:, 0:256],
                    wtile[:, 0:128],
                    wtile[:, 0:256],
                    start=True,
                    stop=True,
                    skip_group_check=True,
                )

        warm(16)

        warm_sb = cpool.tile([128, 2], F32, name="warm", tag="warm")
        nc.vector.memset(warm_sb[:, 0:1], 0.0)
        nc.scalar.activation(warm_sb[:, 1:2], warm_sb[:, 0:1], Exp)

        m01_sb = cpool.tile([128, 128], BF16, name="m01", tag="m01")
        cs_sb = cpool.tile([128, 528], F32, name="cs", tag="cs")
        bq_sb = cs_sb[:, 0:4]
        bk_sb = cs_sb[:, 4:8]
        bv_sb = cs_sb[:, 8:528]

        kT = [persist.tile([128, T], BF16, name=f"kT{p}", tag=f"kT{p}") for p in range(4)]
        Vt = [persist.tile([128, 520], BF16, name=f"V{i}", tag=f"V{i}") for i in range(TT)]

        def pv_step(es, ki, base, yqs, p, first, last):
            r = ki - base
            vs = max(0, 128 * r)
            for h2 in (0, 1):
                hl = 2 * p + h2
                nc.tensor.matmul(
                    yqs[h2][:, vs:512],
                    Vt[ki][:, 65 * hl : 65 * hl + 65],
                    es[:, 512 * h2 + vs : 512 * h2 + 512],
                    start=first,
                    stop=last,
                    skip_group_check=True,
                )

        wq_sb = wpool.tile([128, CT, 512], BF16, name="wq", tag="wq")
        wk_sb = wpool.tile([128, CT, 512], BF16, name="wk", tag="wk")
        wv_sb = wpool.tile([128, CT, 520], BF16, name="wv", tag="wv")
        wp_sb = wpool.tile([128, 4, 1024], BF16, name="wp", tag="wp")

        def dma_x(t):
            xt = xpool.tile([128, CT, 512], BF16, name="xt", tag="xt")
            dma(out=xt[:], in_=d["x8"].ap()[:, 4096 * t : 4096 * t + 4096])
            return xt

        # chunk-0 inputs in quarter-chunks so the first psum group's matmuls
        # can start while the rest streams; consts (biases) must land before
        # the first q/k groups' tensor_scalar consumers
        xt0 = xpool.tile([128, CT, 512], BF16, name="xt", tag="xt")
        for qtr in range(4):
            lo, hi = 1024 * qtr, 1024 * qtr + 1024
            dma(out=wq_sb[:, 2 * qtr : 2 * qtr + 2, :], in_=d["wq"].ap()[:, lo:hi])
            dma(out=xt0[:, 2 * qtr : 2 * qtr + 2, :], in_=d["x8"].ap()[:, lo:hi])
            if qtr == 1:
                # biases land before the first q-group's consumer needs them
                dma(out=cs_sb[:], in_=d["consts"].ap())
        dma(out=wk_sb[:], in_=d["wk"].ap())
        dma(out=wv_sb[:], in_=d["wv"].ap())
        xts = {0: xt0, 1: dma_x(1)}
        dma(out=m01_sb[:], in_=d["m01"].ap())
        dma(out=wp_sb[:], in_=d["wp"].ap())

        qTc = {}  # chunk -> [4 tiles of [128,512] bf16]
        yTc = {}  # chunk -> [4 tiles of [128,512] bf16]
        stage = {}  # chunk -> [128, 4, 1024] bf16 staged out rows

        # ---- filler group emitters (each closure emits one psum group) ----
        def qk_group(t, p, iw, w_sb=None, mid=None):
            def g(w_sb=w_sb):
                xt = xts[t]
                ps = fps.tile([128, 512], F32, name="fps", tag="fps")
                for ct in range(CT):
                    nc.tensor.matmul(
                        ps[:],
                        w_sb[:, ct, 128 * p : 128 * p + 128],
                        xt[:, ct, :],
                        start=(ct == 0),
                        stop=(ct == CT - 1),
                        skip_group_check=True,
                    )
                    if mid is not None and ct in (1, 3, 5):
                        mid()
                if iw == 0:
                    nc.vector.tensor_scalar(
                        qTc[t][p][:],
                        ps[:],
                        0.125,
                        bq_sb[:, p : p + 1],
                        mult,
                        add,
                    )
                else:
                    nc.vector.tensor_scalar(
                        kT[p][:, 512 * t : 512 * t + 512],
                        ps[:],
                        1.0,
                        bk_sb[:, p : p + 1],
                        mult,
                        add,
                    )

            return g

        def v_group(t, tt, qd):
            def g():
                xt = xts[t]
                ps = fps.tile([128, 512], F32, name="fps", tag="fps")
                for ct in range(CT):
                    nc.tensor.matmul(
                        ps[:, 0:260],
                        xt[:, ct, 128 * tt : 128 * tt + 128],
                        wv_sb[:, ct, 260 * qd : 260 * qd + 260],
                        start=(ct == 0),
                        stop=(ct == CT - 1),
                        skip_group_check=True,
                    )
                nc.vector.tensor_tensor(
                    Vt[4 * t + tt][:, 260 * qd : 260 * qd + 260],
                    ps[:, 0:260],
                    bv_sb[:, 260 * qd : 260 * qd + 260],
                    add,
                )

            return g

        def qkv_groups(t, first=False):
            gs = []
            for p in range(4):
                mid = (lambda: warm(5)) if (first and p == 0) else None
                gs.append(qk_group(t, p, 0, wq_sb, mid=mid))
            for p in range(4):
                gs.append(qk_group(t, p, 1, wk_sb))
            for tt in range(4):
                for qd in (0, 1):
                    gs.append(v_group(t, tt, qd))
            return gs

        def proj_group(t, ttl, cc, eng="act"):
            def g():
                ps = fps.tile([128, 512], F32, name="fps", tag="fps")
                for pp in range(4):
                    nc.tensor.matmul(
                        ps[:],
                        yTc[t][pp][:, 128 * ttl : 128 * ttl + 128],
                        wp_sb[:, pp, 512 * cc : 512 * cc + 512],
                        start=(pp == 0),
                        stop=(pp == 3),
                        skip_group_check=True,
                    )
                if t == TC - 1:
                    # epilogue: per-group DMA keeps the drain tail short;
                    # copies alternate ACT/DVE so psum banks free 2x faster
                    row = 512 * t + 128 * ttl
                    ob = opool.tile([128, 512], BF16, name="ob", tag="ob")
                    if (ttl + cc) % 2 == 0:
                        nc.scalar.copy(ob[:], ps[:])
                    else:
                        nc.vector.tensor_copy(out=ob[:], in_=ps[:])
                    dma(
                        out=d["out"].ap()[row : row + 128, 512 * cc : 512 * cc + 512],
                        in_=ob[:],
                    )
                    return
                if t not in stage:
                    stage[t] = stpool.tile(
                        [128, 4, 1024], BF16, name=f"st{t}", tag="st"
                    )
                st = stage[t]
                if eng == "act":
                    nc.scalar.copy(st[:, ttl, 512 * cc : 512 * cc + 512], ps[:])
                else:
                    nc.vector.tensor_copy(
                        out=st[:, ttl, 512 * cc : 512 * cc + 512], in_=ps[:]
                    )
                if ttl == 3 and cc == 1:
                    dma(
                        out=d["out"]
                        .ap()[512 * t : 512 * t + 512, :]
                        .rearrange("(tt p) n -> p tt n", p=128),
                        in_=st[:],
                    )

            return g

        def proj_groups(t, eng="act"):
            return [
                proj_group(t, ttl, cc, eng)
                for ttl in range(4)
                for cc in (0, 1)
            ]

        def flush_tail(yq_t, p, t, sliced=False):
            if not sliced:
                # ACT copies the raw rows out (frees the yq psum bank for the
                # next pair's P@V quickly; ACT has slack at pair ends where
                # the trailing exps are small), DVE normalizes the sbuf copy
                # off the psum-critical path
                for h2 in (0, 1):
                    pr = 64 * h2
                    rr = rcpool.tile([1, 512], F32, name="rr", tag="rr")
                    nc.vector.reciprocal(rr[:], yq_t[h2][64:65, :])
                    rcb = rcpool.tile([128, 512], F32, name="rcb", tag="rcb")
                    nc.gpsimd.partition_broadcast(rcb[:], rr[:])
                    nc.scalar.copy(yTc[t][p][pr : pr + 64, :], yq_t[h2][0:64, :])
                    nc.vector.tensor_tensor(
                        yTc[t][p][pr : pr + 64, :],
                        yTc[t][p][pr : pr + 64, :],
                        rcb[pr : pr + 64, :],
                        mult,
                    )
                return
            # final flush: 128-col slices so the first epilogue out-proj
            # group starts as soon as its slice is normalized
            rcbs = [
                rclpool.tile([128, 512], F32, name=f"rcb{h2}", tag=f"rcbs{h2}")
                for h2 in (0, 1)
            ]
            rrs = [
                rclpool.tile([1, 512], F32, name=f"rrs{h2}", tag=f"rrs{h2}")
                for h2 in (0, 1)
            ]
            for sl in range(4):
                c0 = 128 * sl
                for h2 in (0, 1):
                    pr = 64 * h2
                    nc.vector.reciprocal(
                        rrs[h2][:, c0 : c0 + 128], yq_t[h2][64:65, c0 : c0 + 128]
                    )
                    nc.gpsimd.partition_broadcast(
                        rcbs[h2][:, c0 : c0 + 128], rrs[h2][:, c0 : c0 + 128]
                    )
                    nc.vector.tensor_tensor(
                        yTc[t][p][pr : pr + 64, c0 : c0 + 128],
                        yq_t[h2][0:64, c0 : c0 + 128],
                        rcbs[h2][pr : pr + 64, c0 : c0 + 128],
                        mult,
                    )

        # ---- prologue: qkv for chunk 0 ----
        qTc[0] = [
            qpool.tile([128, 512], BF16, name=f"qT{p}", tag=f"qT{p}") for p in range(4)
        ]
        for g in qkv_groups(0, first=True):
            g()

        # ---- fused chunk pipeline ----
        for t in range(TC):
            # stage next chunks' inputs and allocate next-gen tiles
            if t + 2 < TC:
                xts[t + 2] = dma_x(t + 2)
            filler = []
            if t + 1 < TC:
                qTc[t + 1] = [
                    qpool.tile([128, 512], BF16, name=f"qT{p}", tag=f"qT{p}")
                    for p in range(4)
                ]
                filler += qkv_groups(t + 1)
            # all mid-stream out-proj filler rides in the last chunk, where
            # the exp load is heaviest and the spine alone cannot feed PE
            if t == 3:
                filler += (
                    proj_groups(0, eng="dve")
                    + proj_groups(1, eng="dve")
                    + proj_groups(2, eng="dve")
                )
            yTc[t] = [
                ypool.tile([128, 512], BF16, name=f"yT{p}", tag=f"yT{p}")
                for p in range(4)
            ]

            nki = 4 * t + 4
            nsteps = 4 * nki
            quota = len(filler) / nsteps
            acc = 0.0
            si = 0
            fi = 0
            pending = None  # (yqs, p) flush deferred to overlap next p's S work
            for p in range(4):
                yqs = [
                    psy.tile([65, 512], F32, name=f"yq{h2}", tag=f"yq{h2}")
                    for h2 in (0, 1)
                ]
                if pending is not None:
                    flush_tail(pending[0], pending[1], t)
                    pending = None
                    if fi < len(filler):
                        filler[fi]()
                        fi += 1
                        acc -= 1.0
                es_blk = [None] * nki
                kis = list(range(nki))
                for j, ki in enumerate(kis):
                    r = ki - 4 * t
                    off = 128 * r if r >= 1 else 0
                    # S^T block (both heads in one psum tile), trimmed to the
                    # causally reachable q-columns
                    sblk = pss.tile([128, 1024], F32, name="sblk", tag="sblk")
                    for h2 in (0, 1):
                        pr = 64 * h2
                        nc.tensor.matmul(
                            sblk[:, 512 * h2 + off : 512 * h2 + 512],
                            kT[p][pr : pr + 64, 128 * ki : 128 * ki + 128],
                            qTc[t][p][pr : pr + 64, off:512],
                            start=True,
                            stop=True,
                        )
                    # deferred P@V of the block from 2 steps ago keeps PE 2
                    # steps ahead of the exp->mask chain
                    if j >= 2:
                        kv = kis[j - 2]
                        pv_step(
                            es_blk[kv], kv, 4 * t, yqs, p, j == 2, j - 2 == nki - 1
                        )
                    # filler: qkv(t+1) / out-proj psum groups ride the gaps
                    si += 1
                    if t == TC - 1:
                        acc += quota * 2.0 * si / nsteps
                    else:
                        acc += quota
                    while acc >= 1.0 and fi < len(filler):
                        filler[fi]()
                        fi += 1
                        acc -= 1.0
                    es = espool.tile([128, 1024], BF16, name="es", tag="es")
                    if r >= 1:
                        # exp only the reachable region of each half, one
                        # instruction via a [512,2] middle dim over the halves
                        w = 512 - 128 * r
                        sv = sblk[:, 128 * r : 128 * r + w]
                        s2 = bass.AP(
                            tensor=sv.tensor,
                            offset=sv.offset,
                            ap=[list(sv.ap[0]), [512, 2], list(sv.ap[1])],
                        )
                        ev = es[:, 128 * r : 128 * r + w]
                        e2 = bass.AP(
                            tensor=ev.tensor,
                            offset=ev.offset,
                            ap=[list(ev.ap[0]), [512, 2], list(ev.ap[1])],
                        )
                        nc.scalar.activation(e2, s2, Exp)
                    else:
                        nc.scalar.activation(es[:], sblk[:], Exp)
                    if r >= 0:
                        for h2 in (0, 1):
                            c0 = 512 * h2 + 128 * r
                            nc.vector.tensor_tensor(
                                es[:, c0 : c0 + 128],
                                es[:, c0 : c0 + 128],
                                m01_sb[:],
                                mult,
                            )
                    es_blk[ki] = es
                pv_step(es_blk[nki - 2], nki - 2, 4 * t, yqs, p, False, False)
                pv_step(es_blk[nki - 1], nki - 1, 4 * t, yqs, p, False, True)
                pending = (yqs, p)
            flush_tail(pending[0], pending[1], t, sliced=(t == TC - 1))
            pending = None
            # drain any left-over filler
            while fi < len(filler):
                filler[fi]()
                fi += 1

        # ---- epilogue: out-projection of the last chunk ----
        for g in proj_groups(TC - 1):
            g()


def _build():
    nc = bacc.Bacc("TRN2", target_bir_lowering=False, debug=False, num_devices=N_CORES)
    d = {
        "x8": nc.dram_tensor("x8", [128, 4 * CT * 512], BF16, kind="ExternalInput"),
        "wq": nc.dram_tensor("wq", [128, CT * 512], BF16, kind="ExternalInput"),
        "wk": nc.dram_tensor("wk", [128, CT * 512], BF16, kind="ExternalInput"),
        "wv": nc.dram_tensor("wv", [128, CT * 520], BF16, kind="ExternalInput"),
        "consts": nc.dram_tensor("consts", [128, 528], F32, kind="ExternalInput"),
        "m01": nc.dram_tensor("m01", [128, 128], BF16, kind="ExternalInput"),
        "wp": nc.dram_tensor("wp", [128, 4 * 1024], BF16, kind="ExternalInput"),
        "out": nc.dram_tensor("out", [T, C], BF16, kind="ExternalOutput"),
    }
    with tile.TileContext(nc) as tcx:
        _emit(nc, tcx, d)
    nc.compile()
    return nc


def _prep_core_inputs(c, x, w_attn, b_attn):
    import ml_dtypes

    bf = ml_dtypes.bfloat16
    g = c % 2
    # x8[p, 4096*t + 512*ct + n] = x[b, 512*t + n, 128*ct + p]
    xT = np.ascontiguousarray(x[c // 2].T)  # [C, T]
    x8 = np.ascontiguousarray(
        xT.reshape(CT, 128, TC, 512).transpose(1, 2, 0, 3).reshape(128, -1)
    ).astype(bf)

    def pack_w(w):  # [C, n] -> [128, CT*n], ct-major per partition
        n = w.shape[1]
        return np.ascontiguousarray(
            w.reshape(CT, 128, n).transpose(1, 0, 2).reshape(128, -1)
        ).astype(bf)

    wq = pack_w(w_attn[:, 512 * g : 512 * g + 512])
    wk = pack_w(w_attn[:, 1024 + 512 * g : 1024 + 512 * g + 512])
    wv_f = np.zeros((C, 520), np.float32)
    consts = np.zeros((128, 528), np.float32)
    for hl in range(8):
        hcol = 2048 + 512 * g + 64 * hl
        wv_f[:, 65 * hl : 65 * hl + 64] = w_attn[:, hcol : hcol + 64]
        consts[:, 8 + 65 * hl : 8 + 65 * hl + 64] = b_attn[hcol : hcol + 64][None, :]
        consts[:, 8 + 65 * hl + 64] = 1.0
    wv = pack_w(wv_f)
    for p in range(4):
        consts[:, p] = b_attn[512 * g + 128 * p : 512 * g + 128 * p + 128] * 0.125
        consts[:, 4 + p] = b_attn[
            1024 + 512 * g + 128 * p : 1024 + 512 * g + 128 * p + 128
        ]
    m01 = (np.arange(128)[:, None] <= np.arange(128)[None, :]).astype(bf)
    return dict(x8=x8, wq=wq, wk=wk, wv=wv, consts=consts, m01=m01)


def make_in_maps(x, w_attn, b_attn, w_proj):
    import ml_dtypes

    bf = ml_dtypes.bfloat16
    x = np.asarray(x, np.float32)
    w_attn = np.asarray(w_attn, np.float32)
    b_attn = np.asarray(b_attn, np.float32)
    w_proj = np.asarray(w_proj, np.float32)
    in_maps = []
    for c in range(N_CORES):
        m = _prep_core_inputs(c, x, w_attn, b_attn)
        g = c % 2
        wp = w_proj[512 * g : 512 * g + 512, :]  # [512, 1024]
        m["wp"] = np.ascontiguousarray(
            wp.reshape(4, 128, 1024).transpose(1, 0, 2).reshape(128, -1)
        ).astype(bf)
        in_maps.append(m)
    return in_maps


def get_nc():
    if "nc" not in _cache:
        _cache["nc"] = _build()
    return _cache["nc"]


def gather(results, b_proj):
    b_proj = np.asarray(b_proj, np.float32)
    full = np.empty((B, T, C), np.float32)
    for b in range(B):
        full[b] = (
            results[2 * b]["out"].astype(np.float32)
            + results[2 * b + 1]["out"].astype(np.float32)
            + b_proj[None, :]
        )
    return full


def kernel(x, w_attn, b_attn, w_proj, b_proj):
    nc = get_nc()
    in_maps = make_in_maps(x, w_attn, b_attn, w_proj)
    res = run_bass_kernel_spmd(nc, in_maps, list(range(N_CORES)))
    return gather(res.results, b_proj)
